# revision 15
# baseline (speedup 1.0000x reference)
"""Deformable conv block (offset conv -> bilinear deform depthwise -> pointwise)
on 8 Trainium2 NeuronCores, data-parallel over batch (2 images/core).

v3: bf16 datapath; 4-corner patch-unit gathers (1 descriptor per pixel-tap,
1536B); dw folded into pre-scaled gather images; 1024-idx gather calls
(HW limit); contiguous idx bounce via a permuted accumulator layout.

Layouts: pixel fields exist twice --
  A (linear, for indices):  idx16[p, k, g] = unit idx of pixel 128g+p
  B (gather order, for weights/acc): acc[p', g_B] holds pixel
      phi(p', g_B) = 128*((8t+u)%32) + 16*(2H+(8t+u)//32) + i
      where H=g_B//8, t=g_B%8, u=p'//16, i=p'%16
"""

import numpy as np
import ml_dtypes

import concourse.bass as bass
import concourse.bacc as bacc
import concourse.tile as tile
from concourse import mybir
from concourse.bass_utils import run_bass_kernel_spmd
from concourse.masks import make_identity

F32 = mybir.dt.float32
BF16 = mybir.dt.bfloat16
I16 = mybir.dt.int16
AF = mybir.AluOpType

B, C, CO, H, W = 16, 192, 384, 64, 64
HW = H * W
K2 = 9
PADG = 4
WG = W + 2 * PADG          # 72
NUNIT = WG * WG            # 5184 patch-units per image
NCORES = 8
BPC = B // NCORES          # 2
WC = W + 2                 # 66 conv-pad
NPX = 32                   # groups of 128 pixels
NI = 1024                  # idxs per gather call (HW limit: >1024 fails)

_cache = {}


def _build():
    if "nc" in _cache:
        return _cache["nc"]
    nc = bacc.Bacc("TRN2", target_bir_lowering=False, debug=False, num_swdge_queues=4)

    xc0 = nc.dram_tensor("xc0", [BPC, 128, WC, WC], BF16, kind="ExternalInput")
    xc1 = nc.dram_tensor("xc1", [BPC, 64, WC, WC], BF16, kind="ExternalInput")
    xg4 = nc.dram_tensor("xg4", [BPC, K2, NUNIT, 4 * C], BF16, kind="ExternalInput")
    woff0 = nc.dram_tensor("woff0", [128, 9, 18], BF16, kind="ExternalInput")
    woff1 = nc.dram_tensor("woff1", [64, 9, 18], BF16, kind="ExternalInput")
    cstT = nc.dram_tensor("cstT", [128, NPX, 18], F32, kind="ExternalInput")
    cstP = nc.dram_tensor("cstP", [128, NPX, 18], F32, kind="ExternalInput")
    wpw0 = nc.dram_tensor("wpw0", [128, CO], BF16, kind="ExternalInput")
    wpw1 = nc.dram_tensor("wpw1", [64, CO], BF16, kind="ExternalInput")
    out_d = nc.dram_tensor("out", [BPC, CO, HW], F32, kind="ExternalOutput")
    idx_dram = nc.dram_tensor("idx_scratch", [16, K2, 256], I16)

    with tile.TileContext(nc) as tc:
        import contextlib
        with contextlib.ExitStack() as ctx:
            singles = ctx.enter_context(tc.tile_pool(name="singles", bufs=1))
            work = ctx.enter_context(tc.tile_pool(name="work", bufs=1))
            fbuf = ctx.enter_context(tc.tile_pool(name="fbuf", bufs=1))
            accp = ctx.enter_context(tc.tile_pool(name="accp", bufs=2))
            wgtp = ctx.enter_context(tc.tile_pool(name="wgtp", bufs=2))
            gbuf = ctx.enter_context(tc.tile_pool(name="gbuf", bufs=2))
            tbuf = ctx.enter_context(tc.tile_pool(name="tbuf", bufs=1))
            obuf = ctx.enter_context(tc.tile_pool(name="obuf", bufs=2))
            ps_off = ctx.enter_context(tc.tile_pool(name="ps_off", bufs=2, space="PSUM"))
            ps_tr = ctx.enter_context(tc.tile_pool(name="ps_tr", bufs=2, space="PSUM"))
            ps_bk = ctx.enter_context(tc.tile_pool(name="ps_bk", bufs=2, space="PSUM"))
            ps_pw = ctx.enter_context(tc.tile_pool(name="ps_pw", bufs=2, space="PSUM"))

            ident = singles.tile([128, 128], F32)
            make_identity(nc, ident[:, :])
            identb = singles.tile([128, 128], BF16, tag="identb")
            nc.vector.tensor_copy(identb[:, :], ident[:, :])
            s_w0 = singles.tile([128, 9, 18], BF16, tag="sw0")
            nc.sync.dma_start(out=s_w0[:, :, :], in_=woff0[:, :, :])
            s_w1 = singles.tile([64, 9, 18], BF16, tag="sw1")
            nc.sync.dma_start(out=s_w1[:, :, :], in_=woff1[:, :, :])
            s_cT = singles.tile([128, NPX, 18], F32, tag="scT")
            nc.sync.dma_start(out=s_cT[:, :, :], in_=cstT[:, :, :])
            s_cP = singles.tile([128, NPX, 18], F32, tag="scP")
            nc.sync.dma_start(out=s_cP[:, :, :], in_=cstP[:, :, :])
            s_p0 = singles.tile([128, CO], BF16, tag="sp0")
            nc.sync.dma_start(out=s_p0[:, :], in_=wpw0[:, :])
            s_p1 = singles.tile([64, CO], BF16, tag="sp1")
            nc.sync.dma_start(out=s_p1[:, :], in_=wpw1[:, :])

            for b in range(BPC):
                s_x0 = work.tile([128, WC, WC], BF16, tag="x0")
                nc.sync.dma_start(out=s_x0[:, :, :], in_=xc0[b])
                s_x1 = work.tile([64, WC, WC], BF16, tag="x1")
                nc.sync.dma_start(out=s_x1[:, :, :], in_=xc1[b])

                # ---- offset conv ----
                off_sb = work.tile([18, HW], F32, tag="off")
                for q in range(8):
                    pch = ps_off.tile([18, 512], F32, tag="offps")
                    mm = 0
                    for s in range(9):
                        dy, dx = s // 3, s % 3
                        for src, wt in ((s_x0, s_w0), (s_x1, s_w1)):
                            nc.tensor.matmul(
                                pch[:, :],
                                wt[:, s, :],
                                src[:, 8 * q + dy:8 * q + dy + 8, dx:dx + 64],
                                start=(mm == 0),
                                stop=(mm == 17),
                            )
                            mm += 1
                    nc.vector.tensor_copy(off_sb[:, 512 * q:512 * (q + 1)], pch[:, :])

                # ---- transpose offsets: A (linear) and B (phi-permuted) ----
                offT = work.tile([128, NPX, 18], F32, tag="offT")
                for t in range(NPX):
                    ptr = ps_tr.tile([128, 18], F32, tag="trp")
                    nc.tensor.transpose(
                        ptr[:, :], off_sb[:, 128 * t:128 * (t + 1)], ident[:18, :18]
                    )
                    nc.vector.tensor_copy(offT[:, t, :], ptr[:, :])
                offP = work.tile([128, NPX, 18], F32, tag="offP")
                for gB in range(NPX):
                    Hc, t = gB // 8, gB % 8
                    off_px = 1024 * (t % 4) + 32 * Hc + 16 * (t // 4)
                    o = off_sb[:, :]
                    src = bass.AP(
                        tensor=o.tensor,
                        offset=o.offset + off_px,
                        ap=[o.ap[0], [128, 8], [1, 16]],
                    )
                    stg = fbuf.tile([18, 128], F32, tag="stg")
                    nc.vector.tensor_copy(stg[:, :], src)
                    ptr2 = ps_tr.tile([128, 18], F32, tag="trp")
                    nc.tensor.transpose(ptr2[:, :], stg[:, :], ident[:18, :18])
                    nc.vector.tensor_copy(offP[:, gB, :], ptr2[:, :])

                # ---- A fields: positions -> floor -> unit idx ----
                pos = fbuf.tile([128, NPX, 18], F32, tag="pos")
                nc.vector.tensor_tensor(pos[:, :, :], offT[:, :, :], s_cT[:, :, :], AF.add)
                nc.vector.tensor_scalar(pos[:, :, :], pos[:, :, :], 130.5, 60.5, AF.min, AF.max)
                fl = fbuf.tile([128, NPX, 18], F32, tag="fl")
                nc.vector.tensor_scalar(fl[:, :, :], pos[:, :, :], 8388608.0, -8388608.0, AF.add, AF.add)
                frac = fbuf.tile([128, NPX, 18], F32, tag="frac")
                nc.vector.tensor_tensor(frac[:, :, :], fl[:, :, :], pos[:, :, :], AF.is_gt)
                nc.vector.tensor_tensor(fl[:, :, :], fl[:, :, :], frac[:, :, :], AF.subtract)
                idxf = fbuf.tile([128, K2, NPX], F32, tag="idxf")
                _if = idxf[:, :, :]
                idxf_v = bass.AP(tensor=_if.tensor, offset=_if.offset,
                                 ap=[_if.ap[0], [1, NPX], [NPX, K2]])
                nc.vector.scalar_tensor_tensor(
                    idxf_v, fl[:, :, 0:9], 72.0, fl[:, :, 9:18], AF.mult, AF.add
                )
                idx16 = fbuf.tile([128, K2, NPX], I16, tag="idx16")
                nc.vector.tensor_scalar(idx16[:, :, :], idxf[:, :, :], -4380.0, None, AF.add)

                # ---- B fields: positions -> frac -> bilinear corner weights ----
                posP = fbuf.tile([128, NPX, 18], F32, tag="pos")
                nc.vector.tensor_tensor(posP[:, :, :], offP[:, :, :], s_cP[:, :, :], AF.add)
                nc.vector.tensor_scalar(posP[:, :, :], posP[:, :, :], 130.5, 60.5, AF.min, AF.max)
                flP = fbuf.tile([128, NPX, 18], F32, tag="fl")
                nc.vector.tensor_scalar(flP[:, :, :], posP[:, :, :], 8388608.0, -8388608.0, AF.add, AF.add)
                fracP = fbuf.tile([128, NPX, 18], F32, tag="frac")
                nc.vector.tensor_tensor(fracP[:, :, :], flP[:, :, :], posP[:, :, :], AF.is_gt)
                nc.vector.tensor_tensor(flP[:, :, :], flP[:, :, :], fracP[:, :, :], AF.subtract)
                nc.vector.tensor_tensor(fracP[:, :, :], posP[:, :, :], flP[:, :, :], AF.subtract)
                g1 = fbuf.tile([128, NPX, 18], F32, tag="g1")
                nc.vector.tensor_scalar(g1[:, :, :], fracP[:, :, :], -1.0, 1.0, AF.mult, AF.add)
                wgtb = wgtp.tile([128, 4, NPX, K2], BF16, tag="wgtb")
                nc.vector.tensor_tensor(wgtb[:, 0], g1[:, :, 0:9], g1[:, :, 9:18], AF.mult)
                nc.vector.tensor_tensor(wgtb[:, 1], g1[:, :, 0:9], fracP[:, :, 9:18], AF.mult)
                nc.vector.tensor_tensor(wgtb[:, 2], fracP[:, :, 0:9], g1[:, :, 9:18], AF.mult)
                nc.vector.tensor_tensor(wgtb[:, 3], fracP[:, :, 0:9], fracP[:, :, 9:18], AF.mult)

                # ---- idx rearrange via DRAM bounce (contiguous runs) ----
                # D[i, k, 32m+g] = idx16[16m+i, k, g]
                base = idx_dram[:, :, :]
                for m in range(8):
                    wrap_out = bass.AP(
                        tensor=base.tensor,
                        offset=base.offset + 32 * m,
                        ap=[[2304, 16], [256, K2], [1, 32]],
                    )
                    nc.sync.dma_start(out=wrap_out, in_=idx16[16 * m:16 * (m + 1), :, :])
                idxw = fbuf.tile([128, K2, 256], I16, tag="idxw")
                rep_in = bass.AP(
                    tensor=base.tensor,
                    offset=base.offset,
                    ap=[[0, 8], [2304, 16], [1, 2304]],
                )
                nc.sync.dma_start(out=idxw[:, :, :], in_=rep_in)

                # ---- gather + combine ----
                acc = accp.tile([128, NPX, C], BF16, tag="acc")
                nc.vector.memset(acc[:, :, :], 0.0)
                xg_b = xg4[b]
                for k in range(K2):
                    for hp in range(2):
                        gt = gbuf.tile([128, 16, 4 * C], BF16, tag="gt")
                        src = bass.AP(
                            tensor=xg_b.tensor,
                            offset=xg_b.offset + k * NUNIT * 4 * C,
                            ap=[[4 * C, NUNIT], [1, 4 * C]],
                        )
                        for sub in range(2):
                            Hc = 2 * hp + sub
                            nc.gpsimd.dma_gather(
                                out_ap=gt[:, 8 * sub:8 * (sub + 1), :],
                                in_ap=src,
                                idxs_ap=idxw[:, k, 64 * Hc:64 * (Hc + 1)],
                                num_idxs=NI,
                                num_idxs_reg=NI,
                                elem_size=4 * C,
                                elem_step=4 * C,
                                queue_num=2 * hp + sub,
                            )
                        t1 = tbuf.tile([128, 16, C], BF16, tag="t1")
                        t2 = tbuf.tile([128, 16, C], BF16, tag="t2")
                        wex = tbuf.tile([128, 4, 16, C], BF16, tag="wex")

                        def wap(j):
                            w = wgtb[:, j, 16 * hp:16 * (hp + 1), k]
                            return bass.AP(
                                tensor=w.tensor,
                                offset=w.offset,
                                ap=[w.ap[0], w.ap[1], [0, C]],
                            )

                        w4 = wgtb[:, :, 16 * hp:16 * (hp + 1), k]
                        w4b = bass.AP(
                            tensor=w4.tensor,
                            offset=w4.offset,
                            ap=[w4.ap[0], w4.ap[1], w4.ap[2], [0, C]],
                        )
                        nc.scalar.copy(wex[:, :, :, :], w4b)
                        nc.vector.tensor_tensor(t1[:, :, :], gt[:, :, 0:C], wex[:, 0], AF.mult)
                        nc.vector.tensor_tensor(t2[:, :, :], gt[:, :, C:2 * C], wex[:, 1], AF.mult)
                        nc.vector.tensor_tensor(t1[:, :, :], t1[:, :, :], t2[:, :, :], AF.add)
                        nc.vector.tensor_tensor(t2[:, :, :], gt[:, :, 2 * C:3 * C], wex[:, 2], AF.mult)
                        nc.vector.tensor_tensor(t1[:, :, :], t1[:, :, :], t2[:, :, :], AF.add)
                        nc.vector.tensor_tensor(t2[:, :, :], gt[:, :, 3 * C:4 * C], wex[:, 3], AF.mult)
                        nc.vector.tensor_tensor(t1[:, :, :], t1[:, :, :], t2[:, :, :], AF.add)
                        a_sl = acc[:, 16 * hp:16 * (hp + 1), :]
                        nc.vector.tensor_tensor(a_sl, a_sl, t1[:, :, :], AF.add)

                # ---- transpose back to c-major (scatter to linear px) ----
                dw0 = work.tile([128, HW], BF16, tag="dw0")
                dw1 = work.tile([64, HW], BF16, tag="dw1")
                for gB in range(NPX):
                    Hc, t = gB // 8, gB % 8
                    off_px = 1024 * (t % 4) + 32 * Hc + 16 * (t // 4)
                    pbk = ps_bk.tile([128, 256], BF16, tag="bk0")
                    nc.tensor.transpose(pbk[:, 0:128], acc[:, gB, 0:128], identb[:, :])
                    d0 = dw0[:, :]
                    dst0 = bass.AP(tensor=d0.tensor, offset=d0.offset + off_px,
                                   ap=[d0.ap[0], [128, 8], [1, 16]])
                    nc.scalar.copy(dst0, pbk[:, 0:128])
                    nc.tensor.transpose(pbk[0:64, 128:256], acc[:, gB, 128:192], identb[:, :])
                    d1 = dw1[:, :]
                    dst1 = bass.AP(tensor=d1.tensor, offset=d1.offset + off_px,
                                   ap=[d1.ap[0], [128, 8], [1, 16]])
                    nc.scalar.copy(dst1, pbk[0:64, 128:256])

                # ---- pointwise conv ----
                for q in range(8):
                    for o in range(3):
                        ppw = ps_pw.tile([128, 512], F32, tag="pw")
                        nc.tensor.matmul(
                            ppw[:, :],
                            s_p0[:, 128 * o:128 * (o + 1)],
                            dw0[:, 512 * q:512 * (q + 1)],
                            start=True,
                            stop=False,
                        )
                        nc.tensor.matmul(
                            ppw[:, :],
                            s_p1[:, 128 * o:128 * (o + 1)],
                            dw1[:, 512 * q:512 * (q + 1)],
                            start=False,
                            stop=True,
                        )
                        osb = obuf.tile([128, 512], F32, tag="osb")
                        nc.scalar.copy(osb[:, :], ppw[:, :])
                        nc.sync.dma_start(
                            out=out_d[b, 128 * o:128 * (o + 1), 512 * q:512 * (q + 1)],
                            in_=osb[:, :],
                        )

    nc.compile()
    _cache["nc"] = nc
    return nc


def _host_prep(x, w_off, b_off, w_dw, w_pw):
    """Shared (weight-derived) tensors + per-core input shards."""
    K = 3
    bf = ml_dtypes.bfloat16
    # conv input, zero-padded by 1, c-major, bf16
    xcp = np.zeros((B, C, WC, WC), bf)
    xcp[:, :, 1:65, 1:65] = x.astype(bf)

    # gather patch-unit images: per tap k, dw-prescaled, zero-padded by PADG;
    # unit u=(y,x) holds the 2x2 patch [img[y,x], img[y,x+1], img[y+1,x],
    # img[y+1,x+1]] (bf16)
    xhwc = np.transpose(x, (0, 2, 3, 1))  # [B,H,W,C] f32
    wdw = w_dw.reshape(C, K2)             # [C, 9]
    xg4 = np.empty((B, K2, NUNIT, 4 * C), bf)
    P = np.zeros((B, WG + 1, WG + 1, C), bf)
    for k in range(K2):
        P[:, PADG:PADG + H, PADG:PADG + W, :] = (
            xhwc * wdw[None, None, None, :, k]).astype(bf)
        win = np.lib.stride_tricks.sliding_window_view(P, (2, 2), axis=(1, 2))
        # win: [B, WG, WG, C, 2, 2] -> [B, WG, WG, 2, 2, C]
        xg4[:, k] = win.transpose(0, 1, 2, 4, 5, 3).reshape(B, NUNIT, 4 * C)

    # offset conv stationaries, output channels reordered to [y taps | x taps]
    perm = [2 * k for k in range(K2)] + [2 * k + 1 for k in range(K2)]
    wo = np.empty((9, C, 18), np.float32)
    for s in range(9):
        dy, dx = s // 3, s % 3
        wo[s] = w_off[perm, :, dy, dx].T  # [C, 18]
    wo = wo.transpose(1, 0, 2).astype(bf)  # [C, 9, 18]

    # pos64 = off + base + ki/kj - 1 + b_off + 64, per pixel
    i = np.arange(HW)
    hh, ww = i // W, i % W
    cst = np.empty((HW, 18), np.float32)
    for k in range(K2):
        ki, kj = k // K, k % K
        cst[:, k] = hh - 1 + ki + b_off[2 * k] + 64.0
        cst[:, 9 + k] = ww - 1 + kj + b_off[2 * k + 1] + 64.0
    cstT = cst.reshape(NPX, 128, 18).transpose(1, 0, 2).copy()  # [128, NPX, 18]
    # B layout: pixel phi(p', gB)
    pp = np.arange(128)[:, None]
    gb = np.arange(NPX)[None, :]
    Hc, t = gb // 8, gb % 8
    u, ii = pp // 16, pp % 16
    v = 8 * t + u
    m = 2 * Hc + v // 32
    g = v % 32
    pxP = 128 * g + 16 * m + ii  # [128, NPX]
    cstP = cst[pxP]              # [128, NPX, 18]

    wpwT = w_pw.T.astype(bf)  # [C, CO]

    shared = {
        "woff0": wo[:128].copy(),
        "woff1": wo[128:].copy(),
        "cstT": cstT.astype(np.float32),
        "cstP": cstP.astype(np.float32),
        "wpw0": wpwT[:128].copy(),
        "wpw1": wpwT[128:].copy(),
    }
    in_maps = []
    for cid in range(NCORES):
        bs = slice(cid * BPC, (cid + 1) * BPC)
        m = dict(shared)
        m["xc0"] = xcp[bs, :128]
        m["xc1"] = xcp[bs, 128:]
        m["xg4"] = xg4[bs]
        in_maps.append(m)
    return in_maps


def kernel(x, w_off, b_off, w_dw, w_pw, _trace=False):
    x = np.asarray(x, np.float32)
    w_off = np.asarray(w_off, np.float32)
    b_off = np.asarray(b_off, np.float32)
    w_dw = np.asarray(w_dw, np.float32)
    w_pw = np.asarray(w_pw, np.float32)

    nc = _build()
    in_maps = _host_prep(x, w_off, b_off, w_dw, w_pw)
    res = run_bass_kernel_spmd(nc, in_maps, core_ids=list(range(NCORES)), trace=_trace)
    out = np.concatenate([r["out"] for r in res.results], axis=0)
    if _trace:
        kernel.last_exec_ns = res.exec_time_ns
    return out.reshape(B, CO, H, W)


# revision 16
# speedup vs baseline: 1.1636x; 1.1636x over previous
"""Deformable conv block (offset conv -> bilinear deform depthwise -> pointwise)
on 8 Trainium2 NeuronCores, data-parallel over batch (2 images/core).

v3: bf16 datapath; 4-corner patch-unit gathers (1 descriptor per pixel-tap,
1536B); dw folded into pre-scaled gather images; 1024-idx gather calls
(HW limit); contiguous idx bounce via a permuted accumulator layout.

Layouts: pixel fields exist twice --
  A (linear, for indices):  idx16[p, k, g] = unit idx of pixel 128g+p
  B (gather order, for weights/acc): acc[p', g_B] holds pixel
      phi(p', g_B) = 128*((8t+u)%32) + 16*(2H+(8t+u)//32) + i
      where H=g_B//8, t=g_B%8, u=p'//16, i=p'%16
"""

import numpy as np
import ml_dtypes

import concourse.bass as bass
import concourse.bacc as bacc
import concourse.tile as tile
from concourse import mybir
from concourse.bass_utils import run_bass_kernel_spmd
from concourse.masks import make_identity

F32 = mybir.dt.float32
BF16 = mybir.dt.bfloat16
I16 = mybir.dt.int16
AF = mybir.AluOpType

B, C, CO, H, W = 16, 192, 384, 64, 64
HW = H * W
K2 = 9
PADG = 4
WG = W + 2 * PADG          # 72
NUNIT = WG * WG            # 5184 patch-units per image
NCORES = 8
BPC = B // NCORES          # 2
WC = W + 2                 # 66 conv-pad
NPX = 32                   # groups of 128 pixels
NI = 1024                  # idxs per gather call (HW limit: >1024 fails)

_cache = {}


def _build():
    if "nc" in _cache:
        return _cache["nc"]
    nc = bacc.Bacc("TRN2", target_bir_lowering=False, debug=False, num_swdge_queues=4)

    xc0 = nc.dram_tensor("xc0", [BPC, 128, WC, WC], BF16, kind="ExternalInput")
    xc1 = nc.dram_tensor("xc1", [BPC, 64, WC, WC], BF16, kind="ExternalInput")
    xg4 = nc.dram_tensor("xg4", [BPC, K2, NUNIT, 4 * C], BF16, kind="ExternalInput")
    woff0 = nc.dram_tensor("woff0", [128, 9, 18], BF16, kind="ExternalInput")
    woff1 = nc.dram_tensor("woff1", [64, 9, 18], BF16, kind="ExternalInput")
    cstT = nc.dram_tensor("cstT", [128, NPX, 18], F32, kind="ExternalInput")
    cstP = nc.dram_tensor("cstP", [128, NPX, 18], F32, kind="ExternalInput")
    wpw0 = nc.dram_tensor("wpw0", [128, CO], BF16, kind="ExternalInput")
    wpw1 = nc.dram_tensor("wpw1", [64, CO], BF16, kind="ExternalInput")
    out_d = nc.dram_tensor("out", [BPC, CO, HW], F32, kind="ExternalOutput")
    idx_dram = nc.dram_tensor("idx_scratch", [16, K2, 256], I16)

    with tile.TileContext(nc) as tc:
        import contextlib
        with contextlib.ExitStack() as ctx:
            singles = ctx.enter_context(tc.tile_pool(name="singles", bufs=1))
            work = ctx.enter_context(tc.tile_pool(name="work", bufs=1))
            fbuf = ctx.enter_context(tc.tile_pool(name="fbuf", bufs=1))
            accp = ctx.enter_context(tc.tile_pool(name="accp", bufs=2))
            wgtp = ctx.enter_context(tc.tile_pool(name="wgtp", bufs=2))
            gbuf = ctx.enter_context(tc.tile_pool(name="gbuf", bufs=2))
            tbuf = ctx.enter_context(tc.tile_pool(name="tbuf", bufs=1))
            obuf = ctx.enter_context(tc.tile_pool(name="obuf", bufs=2))
            ps_off = ctx.enter_context(tc.tile_pool(name="ps_off", bufs=2, space="PSUM"))
            ps_tr = ctx.enter_context(tc.tile_pool(name="ps_tr", bufs=2, space="PSUM"))
            ps_bk = ctx.enter_context(tc.tile_pool(name="ps_bk", bufs=2, space="PSUM"))
            ps_pw = ctx.enter_context(tc.tile_pool(name="ps_pw", bufs=2, space="PSUM"))

            ident = singles.tile([128, 128], F32)
            make_identity(nc, ident[:, :])
            identb = singles.tile([128, 128], BF16, tag="identb")
            nc.vector.tensor_copy(identb[:, :], ident[:, :])
            s_w0 = singles.tile([128, 9, 18], BF16, tag="sw0")
            nc.sync.dma_start(out=s_w0[:, :, :], in_=woff0[:, :, :])
            s_w1 = singles.tile([64, 9, 18], BF16, tag="sw1")
            nc.sync.dma_start(out=s_w1[:, :, :], in_=woff1[:, :, :])
            s_cT = singles.tile([128, NPX, 18], F32, tag="scT")
            nc.sync.dma_start(out=s_cT[:, :, :], in_=cstT[:, :, :])
            s_cP = singles.tile([128, NPX, 18], F32, tag="scP")
            nc.sync.dma_start(out=s_cP[:, :, :], in_=cstP[:, :, :])
            s_p0 = singles.tile([128, CO], BF16, tag="sp0")
            nc.sync.dma_start(out=s_p0[:, :], in_=wpw0[:, :])
            s_p1 = singles.tile([64, CO], BF16, tag="sp1")
            nc.sync.dma_start(out=s_p1[:, :], in_=wpw1[:, :])

            for b in range(BPC):
                s_x0 = work.tile([128, WC, WC], BF16, tag="x0")
                nc.sync.dma_start(out=s_x0[:, :, :], in_=xc0[b])
                s_x1 = work.tile([64, WC, WC], BF16, tag="x1")
                nc.sync.dma_start(out=s_x1[:, :, :], in_=xc1[b])

                # ---- offset conv ----
                off_sb = work.tile([18, HW], F32, tag="off")
                for q in range(8):
                    pch = ps_off.tile([18, 512], F32, tag="offps")
                    mm = 0
                    for s in range(9):
                        dy, dx = s // 3, s % 3
                        for src, wt in ((s_x0, s_w0), (s_x1, s_w1)):
                            nc.tensor.matmul(
                                pch[:, :],
                                wt[:, s, :],
                                src[:, 8 * q + dy:8 * q + dy + 8, dx:dx + 64],
                                start=(mm == 0),
                                stop=(mm == 17),
                            )
                            mm += 1
                    nc.vector.tensor_copy(off_sb[:, 512 * q:512 * (q + 1)], pch[:, :])

                # ---- transpose offsets: A (linear) and B (phi-permuted) ----
                offT = work.tile([128, NPX, 18], F32, tag="offT")
                for t in range(NPX):
                    ptr = ps_tr.tile([128, 18], F32, tag="trp")
                    nc.tensor.transpose(
                        ptr[:, :], off_sb[:, 128 * t:128 * (t + 1)], ident[:18, :18]
                    )
                    nc.vector.tensor_copy(offT[:, t, :], ptr[:, :])
                offP = work.tile([128, NPX, 18], F32, tag="offP")
                for gB in range(NPX):
                    Hc, t = gB // 8, gB % 8
                    off_px = 1024 * (t % 4) + 32 * Hc + 16 * (t // 4)
                    o = off_sb[:, :]
                    src = bass.AP(
                        tensor=o.tensor,
                        offset=o.offset + off_px,
                        ap=[o.ap[0], [128, 8], [1, 16]],
                    )
                    stg = fbuf.tile([18, 128], F32, tag="stg")
                    nc.vector.tensor_copy(stg[:, :], src)
                    ptr2 = ps_tr.tile([128, 18], F32, tag="trp")
                    nc.tensor.transpose(ptr2[:, :], stg[:, :], ident[:18, :18])
                    nc.vector.tensor_copy(offP[:, gB, :], ptr2[:, :])

                # ---- A fields: positions -> floor -> unit idx ----
                pos = fbuf.tile([128, NPX, 18], F32, tag="pos")
                nc.vector.tensor_tensor(pos[:, :, :], offT[:, :, :], s_cT[:, :, :], AF.add)
                nc.vector.tensor_scalar(pos[:, :, :], pos[:, :, :], 130.5, 60.5, AF.min, AF.max)
                fl = fbuf.tile([128, NPX, 18], F32, tag="fl")
                nc.vector.tensor_scalar(fl[:, :, :], pos[:, :, :], 8388608.0, -8388608.0, AF.add, AF.add)
                frac = fbuf.tile([128, NPX, 18], F32, tag="frac")
                nc.vector.tensor_tensor(frac[:, :, :], fl[:, :, :], pos[:, :, :], AF.is_gt)
                nc.vector.tensor_tensor(fl[:, :, :], fl[:, :, :], frac[:, :, :], AF.subtract)
                idxf = fbuf.tile([128, K2, NPX], F32, tag="idxf")
                _if = idxf[:, :, :]
                idxf_v = bass.AP(tensor=_if.tensor, offset=_if.offset,
                                 ap=[_if.ap[0], [1, NPX], [NPX, K2]])
                nc.vector.scalar_tensor_tensor(
                    idxf_v, fl[:, :, 0:9], 72.0, fl[:, :, 9:18], AF.mult, AF.add
                )
                idx16 = fbuf.tile([128, K2, NPX], I16, tag="idx16")
                nc.vector.tensor_scalar(idx16[:, :, :], idxf[:, :, :], -4380.0, None, AF.add)

                # ---- B fields: positions -> frac -> bilinear corner weights ----
                posP = fbuf.tile([128, NPX, 18], F32, tag="pos")
                nc.vector.tensor_tensor(posP[:, :, :], offP[:, :, :], s_cP[:, :, :], AF.add)
                nc.vector.tensor_scalar(posP[:, :, :], posP[:, :, :], 130.5, 60.5, AF.min, AF.max)
                flP = fbuf.tile([128, NPX, 18], F32, tag="fl")
                nc.vector.tensor_scalar(flP[:, :, :], posP[:, :, :], 8388608.0, -8388608.0, AF.add, AF.add)
                fracP = fbuf.tile([128, NPX, 18], F32, tag="frac")
                nc.vector.tensor_tensor(fracP[:, :, :], flP[:, :, :], posP[:, :, :], AF.is_gt)
                nc.vector.tensor_tensor(flP[:, :, :], flP[:, :, :], fracP[:, :, :], AF.subtract)
                nc.vector.tensor_tensor(fracP[:, :, :], posP[:, :, :], flP[:, :, :], AF.subtract)
                g1 = fbuf.tile([128, NPX, 18], F32, tag="g1")
                nc.vector.tensor_scalar(g1[:, :, :], fracP[:, :, :], -1.0, 1.0, AF.mult, AF.add)
                wgtb = wgtp.tile([128, 4, NPX, K2], BF16, tag="wgtb")
                nc.vector.tensor_tensor(wgtb[:, 0], g1[:, :, 0:9], g1[:, :, 9:18], AF.mult)
                nc.vector.tensor_tensor(wgtb[:, 1], g1[:, :, 0:9], fracP[:, :, 9:18], AF.mult)
                nc.vector.tensor_tensor(wgtb[:, 2], fracP[:, :, 0:9], g1[:, :, 9:18], AF.mult)
                nc.vector.tensor_tensor(wgtb[:, 3], fracP[:, :, 0:9], fracP[:, :, 9:18], AF.mult)

                # ---- idx rearrange via DRAM bounce (contiguous runs) ----
                # D[i, k, 32m+g] = idx16[16m+i, k, g]
                base = idx_dram[:, :, :]
                for m in range(8):
                    wrap_out = bass.AP(
                        tensor=base.tensor,
                        offset=base.offset + 32 * m,
                        ap=[[2304, 16], [256, K2], [1, 32]],
                    )
                    nc.sync.dma_start(out=wrap_out, in_=idx16[16 * m:16 * (m + 1), :, :])
                idxw = fbuf.tile([128, K2, 256], I16, tag="idxw")
                rep_in = bass.AP(
                    tensor=base.tensor,
                    offset=base.offset,
                    ap=[[0, 8], [2304, 16], [1, 2304]],
                )
                nc.sync.dma_start(out=idxw[:, :, :], in_=rep_in)

                # ---- gather + combine ----
                acc = accp.tile([128, NPX, C], BF16, tag="acc")
                nc.vector.memset(acc[:, :, :], 0.0)
                xg_b = xg4[b]
                for k in range(K2):
                    for hp in range(2):
                        gt = gbuf.tile([128, 16, 4 * C], BF16, tag="gt")
                        src = bass.AP(
                            tensor=xg_b.tensor,
                            offset=xg_b.offset + k * NUNIT * 4 * C,
                            ap=[[4 * C, NUNIT], [1, 4 * C]],
                        )
                        for sub in range(2):
                            Hc = 2 * hp + sub
                            nc.gpsimd.dma_gather(
                                out_ap=gt[:, 8 * sub:8 * (sub + 1), :],
                                in_ap=src,
                                idxs_ap=idxw[:, k, 64 * Hc:64 * (Hc + 1)],
                                num_idxs=NI,
                                num_idxs_reg=NI,
                                elem_size=4 * C,
                                elem_step=4 * C,
                                queue_num=2 * hp + sub,
                            )
                        t1 = tbuf.tile([128, 16, C], BF16, tag="t1")
                        t2 = tbuf.tile([128, 16, C], BF16, tag="t2")
                        wex = tbuf.tile([128, 4, 16, C], BF16, tag="wex")

                        def wap(j):
                            w = wgtb[:, j, 16 * hp:16 * (hp + 1), k]
                            return bass.AP(
                                tensor=w.tensor,
                                offset=w.offset,
                                ap=[w.ap[0], w.ap[1], [0, C]],
                            )

                        for j in range(4):
                            nc.scalar.copy(wex[:, j], wap(j))
                        nc.vector.tensor_tensor(t1[:, :, :], gt[:, :, 0:C], wex[:, 0], AF.mult)
                        nc.vector.tensor_tensor(t2[:, :, :], gt[:, :, C:2 * C], wex[:, 1], AF.mult)
                        nc.vector.tensor_tensor(t1[:, :, :], t1[:, :, :], t2[:, :, :], AF.add)
                        nc.vector.tensor_tensor(t2[:, :, :], gt[:, :, 2 * C:3 * C], wex[:, 2], AF.mult)
                        nc.vector.tensor_tensor(t1[:, :, :], t1[:, :, :], t2[:, :, :], AF.add)
                        nc.vector.tensor_tensor(t2[:, :, :], gt[:, :, 3 * C:4 * C], wex[:, 3], AF.mult)
                        nc.vector.tensor_tensor(t1[:, :, :], t1[:, :, :], t2[:, :, :], AF.add)
                        a_sl = acc[:, 16 * hp:16 * (hp + 1), :]
                        nc.vector.tensor_tensor(a_sl, a_sl, t1[:, :, :], AF.add)

                # ---- transpose back to c-major (scatter to linear px) ----
                dw0 = work.tile([128, HW], BF16, tag="dw0")
                dw1 = work.tile([64, HW], BF16, tag="dw1")
                for gB in range(NPX):
                    Hc, t = gB // 8, gB % 8
                    off_px = 1024 * (t % 4) + 32 * Hc + 16 * (t // 4)
                    pbk = ps_bk.tile([128, 256], BF16, tag="bk0")
                    nc.tensor.transpose(pbk[:, 0:128], acc[:, gB, 0:128], identb[:, :])
                    d0 = dw0[:, :]
                    dst0 = bass.AP(tensor=d0.tensor, offset=d0.offset + off_px,
                                   ap=[d0.ap[0], [128, 8], [1, 16]])
                    nc.scalar.copy(dst0, pbk[:, 0:128])
                    nc.tensor.transpose(pbk[0:64, 128:256], acc[:, gB, 128:192], identb[:, :])
                    d1 = dw1[:, :]
                    dst1 = bass.AP(tensor=d1.tensor, offset=d1.offset + off_px,
                                   ap=[d1.ap[0], [128, 8], [1, 16]])
                    nc.scalar.copy(dst1, pbk[0:64, 128:256])

                # ---- pointwise conv ----
                for q in range(8):
                    for o in range(3):
                        ppw = ps_pw.tile([128, 512], F32, tag="pw")
                        nc.tensor.matmul(
                            ppw[:, :],
                            s_p0[:, 128 * o:128 * (o + 1)],
                            dw0[:, 512 * q:512 * (q + 1)],
                            start=True,
                            stop=False,
                        )
                        nc.tensor.matmul(
                            ppw[:, :],
                            s_p1[:, 128 * o:128 * (o + 1)],
                            dw1[:, 512 * q:512 * (q + 1)],
                            start=False,
                            stop=True,
                        )
                        osb = obuf.tile([128, 512], F32, tag="osb")
                        nc.scalar.copy(osb[:, :], ppw[:, :])
                        nc.sync.dma_start(
                            out=out_d[b, 128 * o:128 * (o + 1), 512 * q:512 * (q + 1)],
                            in_=osb[:, :],
                        )

    nc.compile()
    _cache["nc"] = nc
    return nc


def _host_prep(x, w_off, b_off, w_dw, w_pw):
    """Shared (weight-derived) tensors + per-core input shards."""
    K = 3
    bf = ml_dtypes.bfloat16
    # conv input, zero-padded by 1, c-major, bf16
    xcp = np.zeros((B, C, WC, WC), bf)
    xcp[:, :, 1:65, 1:65] = x.astype(bf)

    # gather patch-unit images: per tap k, dw-prescaled, zero-padded by PADG;
    # unit u=(y,x) holds the 2x2 patch [img[y,x], img[y,x+1], img[y+1,x],
    # img[y+1,x+1]] (bf16)
    xhwc = np.transpose(x, (0, 2, 3, 1))  # [B,H,W,C] f32
    wdw = w_dw.reshape(C, K2)             # [C, 9]
    xg4 = np.empty((B, K2, NUNIT, 4 * C), bf)
    P = np.zeros((B, WG + 1, WG + 1, C), bf)
    for k in range(K2):
        P[:, PADG:PADG + H, PADG:PADG + W, :] = (
            xhwc * wdw[None, None, None, :, k]).astype(bf)
        win = np.lib.stride_tricks.sliding_window_view(P, (2, 2), axis=(1, 2))
        # win: [B, WG, WG, C, 2, 2] -> [B, WG, WG, 2, 2, C]
        xg4[:, k] = win.transpose(0, 1, 2, 4, 5, 3).reshape(B, NUNIT, 4 * C)

    # offset conv stationaries, output channels reordered to [y taps | x taps]
    perm = [2 * k for k in range(K2)] + [2 * k + 1 for k in range(K2)]
    wo = np.empty((9, C, 18), np.float32)
    for s in range(9):
        dy, dx = s // 3, s % 3
        wo[s] = w_off[perm, :, dy, dx].T  # [C, 18]
    wo = wo.transpose(1, 0, 2).astype(bf)  # [C, 9, 18]

    # pos64 = off + base + ki/kj - 1 + b_off + 64, per pixel
    i = np.arange(HW)
    hh, ww = i // W, i % W
    cst = np.empty((HW, 18), np.float32)
    for k in range(K2):
        ki, kj = k // K, k % K
        cst[:, k] = hh - 1 + ki + b_off[2 * k] + 64.0
        cst[:, 9 + k] = ww - 1 + kj + b_off[2 * k + 1] + 64.0
    cstT = cst.reshape(NPX, 128, 18).transpose(1, 0, 2).copy()  # [128, NPX, 18]
    # B layout: pixel phi(p', gB)
    pp = np.arange(128)[:, None]
    gb = np.arange(NPX)[None, :]
    Hc, t = gb // 8, gb % 8
    u, ii = pp // 16, pp % 16
    v = 8 * t + u
    m = 2 * Hc + v // 32
    g = v % 32
    pxP = 128 * g + 16 * m + ii  # [128, NPX]
    cstP = cst[pxP]              # [128, NPX, 18]

    wpwT = w_pw.T.astype(bf)  # [C, CO]

    shared = {
        "woff0": wo[:128].copy(),
        "woff1": wo[128:].copy(),
        "cstT": cstT.astype(np.float32),
        "cstP": cstP.astype(np.float32),
        "wpw0": wpwT[:128].copy(),
        "wpw1": wpwT[128:].copy(),
    }
    in_maps = []
    for cid in range(NCORES):
        bs = slice(cid * BPC, (cid + 1) * BPC)
        m = dict(shared)
        m["xc0"] = xcp[bs, :128]
        m["xc1"] = xcp[bs, 128:]
        m["xg4"] = xg4[bs]
        in_maps.append(m)
    return in_maps


def kernel(x, w_off, b_off, w_dw, w_pw, _trace=False):
    x = np.asarray(x, np.float32)
    w_off = np.asarray(w_off, np.float32)
    b_off = np.asarray(b_off, np.float32)
    w_dw = np.asarray(w_dw, np.float32)
    w_pw = np.asarray(w_pw, np.float32)

    nc = _build()
    in_maps = _host_prep(x, w_off, b_off, w_dw, w_pw)
    res = run_bass_kernel_spmd(nc, in_maps, core_ids=list(range(NCORES)), trace=_trace)
    out = np.concatenate([r["out"] for r in res.results], axis=0)
    if _trace:
        kernel.last_exec_ns = res.exec_time_ns
    return out.reshape(B, CO, H, W)
